# revision 10
# baseline (speedup 1.0000x reference)
"""Trainium2 Bass kernel for batched self-attention with q=k=v (BMMAttention).

Problem: hidden_states [16, 2048, 128] f32; out = softmax(x @ x^T) @ x per batch.

Sharding: pure data parallel — 2 batches per core on 8 cores, no collectives.

Per-batch algorithm (S=2048, D=128), designed around softmax row-offset
invariance.  Let q_t = ||x_t||^2 (the score diagonal, which is also the row
max for gaussian-ish inputs by a huge margin).  Using the row offset q_s:

  out[s] = (v_s * exp(S_ss - q_s) + sum_{t!=s} v_t exp(S_ts - q_s)) / den_s
         = (v_s * 1 + corr^T[:,s] * exp(C - q_s)) / den_s

where corr^T[d,s] = sum_{t!=s} (u_t v_td) exp(S_ts - q_t), u_t = exp(q_t - C),
den_s = 1 + sum_{t!=s} exp(S_ts - q_s).  (S symmetric since q=k.)

This form needs NO transposes of the 2048x2048 score matrix: the exp tiles
are produced in [t-row, s-col] layout and consumed directly as the moving
operand of the correction matmul (contract over t = partitions), giving the
output transposed [d, s], which is cheap to transpose back (16 PE transposes
of 128x128).

Engine/dtype choices:
 - QK^T on TensorE in float32r (full rate at N=512).
 - score diagonal forced to -4e4 in PSUM pre-exp, so exp underflows to exactly
   0 there; the diagonal softmax weight is instead applied exactly (weight 1)
   on the f32 path.  This removes any sensitivity to fp32r rounding of the
   diagonal and makes the bf16 correction path affect only ~e^-40 terms.
 - exp on ScalarE reading PSUM [128,1024], writing bf16 P tiles with fused
   row-sum (accum_out) for the denominator.
 - correction matmul in bf16 at N=512 (full rate).
 - dominant path (v_s * 1/den) stays entirely in f32.
"""

import numpy as np

import concourse.bacc as bacc
import concourse.bass as bass
import concourse.mybir as mybir
import concourse.tile as tile
from concourse.bass import ds, ts
from concourse.bass_utils import run_bass_kernel_spmd
from concourse.masks import make_identity

B, S, D = 16, 2048, 128
NCORES = 8
BPC = B // NCORES          # batches per core
KB = S // 128              # 16 row blocks
NJ = S // 512              # 4 column tiles of 512
C_OFF = 128.0              # u = exp(q - C); q ~ chi2(128) so this centers it
DIAG_KILL = -40000.0       # score diagonal becomes ~-4e4 -> exp == 0.0 exactly

F32 = mybir.dt.float32
F32R = mybir.dt.float32r
BF16 = mybir.dt.bfloat16
EXP = mybir.ActivationFunctionType.Exp
ADD = mybir.AluOpType.add
MULT = mybir.AluOpType.mult
AX_X = mybir.AxisListType.X


def build_program():
    nc = bacc.Bacc(
        "TRN2",
        target_bir_lowering=False,
        debug=False,
        num_devices=NCORES,
    )
    x_dram = nc.dram_tensor("x", [BPC, S, D], F32, kind="ExternalInput")
    o_dram = nc.dram_tensor("out", [BPC, S, D], F32, kind="ExternalOutput")

    with tile.TileContext(nc) as tc:
        with (
            tc.tile_pool(name="const", bufs=1) as constp,
            tc.tile_pool(name="io", bufs=2) as iop,
            tc.tile_pool(name="pp", bufs=18) as pp,
            tc.tile_pool(name="small", bufs=2) as smallp,
            tc.tile_pool(name="t1p", bufs=4) as t1p,
            tc.tile_pool(name="qkps", bufs=2, space="PSUM") as qkps,
            tc.tile_pool(name="avps", bufs=4, space="PSUM") as avps,
        ):
            ident = constp.tile([128, 128], F32)
            make_identity(nc, ident[:])
            cneg = constp.tile([128, 1], F32, tag="cneg")
            nc.gpsimd.memset(cneg[:], -C_OFF)
            cpos = constp.tile([128, 1], F32, tag="cpos")
            nc.gpsimd.memset(cpos[:], C_OFF)

            for b in range(BPC):
                # ---- load x in natural layout: x_nat[p, k*128+d] = x[k*128+p, d]
                x_nat = iop.tile([128, S], F32, tag="x_nat")
                nc.sync.dma_start(
                    out=x_nat[:].rearrange("p (k d) -> p k d", d=128),
                    in_=x_dram.ap()[b].rearrange("(k p) d -> p k d", p=128),
                )

                # ---- xT[d, s] via PE transposes of the 16 [128,128] chunks
                xT = iop.tile([128, S], F32R, tag="xT")
                for k in range(KB):
                    tp = qkps.tile([128, 128], F32, tag="qk")
                    nc.tensor.transpose(tp[:], x_nat[:, ts(k, 128)], ident[:])
                    nc.vector.tensor_copy(out=xT[:, ts(k, 128)], in_=tp[:])

                # ---- q_t = ||x_t||^2 in natural layout: sq[p, k]
                xsq = iop.tile([128, S], F32, tag="xsq")
                nc.vector.tensor_mul(xsq[:], x_nat[:], x_nat[:])
                sq = smallp.tile([128, KB], F32, tag="sq")
                nc.vector.tensor_reduce(
                    out=sq[:],
                    in_=xsq[:].rearrange("p (k d) -> p k d", d=128),
                    axis=AX_X,
                    op=ADD,
                )
                negsq = smallp.tile([128, KB], F32, tag="negsq")
                nc.vector.tensor_scalar_mul(negsq[:], sq[:], -1.0)
                # u = exp(q - C), ru = exp(C - q)
                u = smallp.tile([128, KB], F32, tag="u")
                nc.scalar.activation(u[:], sq[:], EXP, bias=cneg[:], scale=1.0)
                ru = smallp.tile([128, KB], F32, tag="ru")
                nc.scalar.activation(ru[:], sq[:], EXP, bias=cpos[:], scale=-1.0)

                # ---- vt[t, d] = u_t * v[t, d]  (bf16, natural layout)
                vt = iop.tile([128, S], BF16, tag="vt")
                for k in range(KB):
                    nc.vector.tensor_scalar_mul(
                        vt[:, ts(k, 128)], x_nat[:, ts(k, 128)], u[:, k : k + 1]
                    )

                # ---- main stream: QK^T -> diag-kill -> exp -> AV accumulation
                s1 = smallp.tile([128, 2 * KB], F32, tag="s1")
                av = [
                    avps.tile([128, 512], F32, tag="av", name=f"av{j}")
                    for j in range(NJ)
                ]
                xTr = xT[:]
                for k in range(KB):
                    Pk = pp.tile([128, S], BF16, tag="P")
                    for c in range(2):
                        qk = qkps.tile([128, 1024], F32, tag="qk")
                        for jj in range(2):
                            nc.tensor.matmul(
                                qk[:, ts(jj, 512)],
                                lhsT=xTr[:, ts(k, 128)],
                                rhs=xTr[:, ds(c * 1024 + jj * 512, 512)],
                                start=True,
                                stop=True,
                            )
                        if k // 8 == c:
                            # kill the score diagonal: S[t,t] += -4e4
                            cw = k * 128 - c * 1024
                            nc.vector.scalar_tensor_tensor(
                                out=qk[:, ds(cw, 128)],
                                in0=ident[:],
                                scalar=DIAG_KILL,
                                in1=qk[:, ds(cw, 128)],
                                op0=MULT,
                                op1=ADD,
                            )
                        nc.scalar.activation(
                            out=Pk[:, ts(c, 1024)],
                            in_=qk[:],
                            func=EXP,
                            bias=negsq[:, k : k + 1],
                            scale=1.0,
                            accum_out=s1[:, 2 * k + c : 2 * k + c + 1],
                        )
                    for j in range(NJ):
                        nc.tensor.matmul(
                            av[j][:],
                            lhsT=vt[:, ts(k, 128)],
                            rhs=Pk[:, ts(j, 512)],
                            start=(k == 0),
                            stop=(k == KB - 1),
                        )

                # ---- denominator: den = 1 + sum of the 32 partial sums
                s12 = smallp.tile([128, KB], F32, tag="s12")
                nc.vector.tensor_reduce(
                    out=s12[:],
                    in_=s1[:].rearrange("p (k c) -> p k c", c=2),
                    axis=AX_X,
                    op=ADD,
                )
                den = smallp.tile([128, KB], F32, tag="den")
                nc.vector.tensor_scalar_add(den[:], s12[:], 1.0)
                r = smallp.tile([128, KB], F32, tag="r")
                nc.vector.reciprocal(r[:], den[:])

                # ---- drain AV groups: transpose corr^T back and combine
                out_nat = iop.tile([128, S], F32, tag="out_nat")
                for j in range(NJ):
                    corrT = iop.tile([128, 512], F32, tag="corrT")
                    nc.vector.tensor_copy(out=corrT[:], in_=av[j][:])
                    for i2 in range(4):
                        i = j * 4 + i2
                        tp2 = qkps.tile([128, 128], F32, tag="qk")
                        nc.tensor.transpose(tp2[:], corrT[:, ts(i2, 128)], ident[:])
                        t1 = t1p.tile([128, 128], F32, tag="t1")
                        nc.vector.scalar_tensor_tensor(
                            out=t1[:],
                            in0=tp2[:],
                            scalar=ru[:, i : i + 1],
                            in1=x_nat[:, ts(i, 128)],
                            op0=MULT,
                            op1=ADD,
                        )
                        nc.vector.tensor_scalar_mul(
                            out_nat[:, ts(i, 128)], t1[:], r[:, i : i + 1]
                        )

                nc.sync.dma_start(
                    out=o_dram.ap()[b].rearrange("(k p) d -> p k d", p=128),
                    in_=out_nat[:].rearrange("p (k d) -> p k d", d=128),
                )
    nc.compile()
    return nc


_PROGRAM = None


def _get_program():
    global _PROGRAM
    if _PROGRAM is None:
        _PROGRAM = build_program()
    return _PROGRAM


def run(hidden_states, trace=False, trace_kwargs=None):
    hs = np.ascontiguousarray(np.asarray(hidden_states, dtype=np.float32))
    assert hs.shape == (B, S, D), hs.shape
    nc = _get_program()
    in_maps = [
        {"x": np.ascontiguousarray(hs[c * BPC : (c + 1) * BPC])}
        for c in range(NCORES)
    ]
    res = run_bass_kernel_spmd(
        nc,
        in_maps,
        core_ids=list(range(NCORES)),
        trace=trace,
        **(trace_kwargs or {}),
    )
    out = np.concatenate([r["out"] for r in res.results], axis=0)
    return out, res


def kernel(hidden_states):
    out, _ = run(hidden_states, trace=False)
    return (out, None)


# revision 12
# speedup vs baseline: 1351.9383x; 1351.9383x over previous
"""Trainium2 Bass kernel for batched self-attention with q=k=v (BMMAttention).

Problem: hidden_states [16, 2048, 128] f32; out = softmax(x @ x^T) @ x per batch.

Sharding: pure data parallel — 2 batches per core on 8 cores, no collectives.

Per-batch algorithm (S=2048, D=128), designed around softmax row-offset
invariance.  Let q_t = ||x_t||^2 (the score diagonal, which is also the row
max for gaussian-ish inputs by a huge margin).  Using the row offset q_s:

  out[s] = (v_s * exp(S_ss - q_s) + sum_{t!=s} v_t exp(S_ts - q_s)) / den_s
         = (v_s * 1 + corr^T[:,s] * exp(C - q_s)) / den_s

where corr^T[d,s] = sum_{t!=s} (u_t v_td) exp(S_ts - q_t), u_t = exp(q_t - C),
den_s = 1 + sum_{t!=s} exp(S_ts - q_s).  (S symmetric since q=k.)

This form needs NO transposes of the 2048x2048 score matrix: the exp tiles
are produced in [t-row, s-col] layout and consumed directly as the moving
operand of the correction matmul (contract over t = partitions), giving the
output transposed [d, s], which is cheap to transpose back (16 PE transposes
of 128x128).

Engine/dtype choices:
 - QK^T on TensorE in float32r (full rate at N=512).
 - score diagonal forced to -4e4 in PSUM pre-exp, so exp underflows to exactly
   0 there; the diagonal softmax weight is instead applied exactly (weight 1)
   on the f32 path.  This removes any sensitivity to fp32r rounding of the
   diagonal and makes the bf16 correction path affect only ~e^-40 terms.
 - exp on ScalarE reading PSUM [128,1024], writing bf16 P tiles with fused
   row-sum (accum_out) for the denominator.
 - correction matmul in bf16 at N=512 (full rate).
 - dominant path (v_s * 1/den) stays entirely in f32.
"""

import numpy as np

import concourse.bacc as bacc
import concourse.bass as bass
import concourse.mybir as mybir
import concourse.tile as tile
from concourse.bass import ds, ts
from concourse.bass_utils import run_bass_kernel_spmd
from concourse.masks import make_identity

B, S, D = 16, 2048, 128
NCORES = 8
BPC = B // NCORES          # batches per core
KB = S // 128              # 16 row blocks
NJ = S // 512              # 4 column tiles of 512
C_OFF = 128.0              # u = exp(q - C); q ~ chi2(128) so this centers it
DIAG_KILL = -40000.0       # score diagonal becomes ~-4e4 -> exp == 0.0 exactly

F32 = mybir.dt.float32
F32R = mybir.dt.float32r
BF16 = mybir.dt.bfloat16
EXP = mybir.ActivationFunctionType.Exp
ADD = mybir.AluOpType.add
MULT = mybir.AluOpType.mult
AX_X = mybir.AxisListType.X


def build_program(loop_n=0):
    nc = bacc.Bacc(
        "TRN2",
        target_bir_lowering=False,
        debug=False,
        num_devices=NCORES,
    )
    x_dram = nc.dram_tensor("x", [BPC, S, D], F32, kind="ExternalInput")
    o_dram = nc.dram_tensor("out", [BPC, S, D], F32, kind="ExternalOutput")

    with tile.TileContext(nc) as tc:
        with (
            tc.tile_pool(name="const", bufs=1) as constp,
            tc.tile_pool(name="io", bufs=2) as iop,
            tc.tile_pool(name="pp", bufs=18) as pp,
            tc.tile_pool(name="small", bufs=2) as smallp,
            tc.tile_pool(name="t1p", bufs=4) as t1p,
            tc.tile_pool(name="qkps", bufs=2, space="PSUM") as qkps,
            tc.tile_pool(name="avps", bufs=4, space="PSUM") as avps,
        ):
            ident = constp.tile([128, 128], F32)
            make_identity(nc, ident[:])
            cneg = constp.tile([128, 1], F32, tag="cneg")
            nc.gpsimd.memset(cneg[:], -C_OFF)
            cpos = constp.tile([128, 1], F32, tag="cpos")
            nc.gpsimd.memset(cpos[:], C_OFF)

            def batch_body(b):
                # ---- load x in natural layout: x_nat[p, k*128+d] = x[k*128+p, d]
                x_nat = iop.tile([128, S], F32, tag="x_nat")
                nc.sync.dma_start(
                    out=x_nat[:].rearrange("p (k d) -> p k d", d=128),
                    in_=x_dram.ap()[b].rearrange("(k p) d -> p k d", p=128),
                )

                # ---- xT[d, s] via PE transposes of the 16 [128,128] chunks
                xT = iop.tile([128, S], F32R, tag="xT")
                for k in range(KB):
                    tp = qkps.tile([128, 128], F32, tag="qk")
                    nc.tensor.transpose(tp[:], x_nat[:, ts(k, 128)], ident[:])
                    nc.vector.tensor_copy(out=xT[:, ts(k, 128)], in_=tp[:])

                # ---- q_t = ||x_t||^2 in natural layout: sq[p, k]
                xsq = iop.tile([128, S], F32, tag="xsq")
                nc.vector.tensor_mul(xsq[:], x_nat[:], x_nat[:])
                sq = smallp.tile([128, KB], F32, tag="sq")
                nc.vector.tensor_reduce(
                    out=sq[:],
                    in_=xsq[:].rearrange("p (k d) -> p k d", d=128),
                    axis=AX_X,
                    op=ADD,
                )
                negsq = smallp.tile([128, KB], F32, tag="negsq")
                nc.vector.tensor_scalar_mul(negsq[:], sq[:], -1.0)
                # u = exp(q - C), ru = exp(C - q)
                u = smallp.tile([128, KB], F32, tag="u")
                nc.scalar.activation(u[:], sq[:], EXP, bias=cneg[:], scale=1.0)
                ru = smallp.tile([128, KB], F32, tag="ru")
                nc.scalar.activation(ru[:], sq[:], EXP, bias=cpos[:], scale=-1.0)

                # ---- vt[t, d] = u_t * v[t, d]  (bf16, natural layout)
                vt = iop.tile([128, S], BF16, tag="vt")
                for k in range(KB):
                    nc.vector.tensor_scalar_mul(
                        vt[:, ts(k, 128)], x_nat[:, ts(k, 128)], u[:, k : k + 1]
                    )

                # ---- main stream: QK^T -> diag-kill -> exp -> AV accumulation
                s1 = smallp.tile([128, 2 * KB], F32, tag="s1")
                av = [
                    avps.tile([128, 512], F32, tag="av", name=f"av{j}")
                    for j in range(NJ)
                ]
                xTr = xT[:]
                for k in range(KB):
                    Pk = pp.tile([128, S], BF16, tag="P")
                    for c in range(2):
                        qk = qkps.tile([128, 1024], F32, tag="qk")
                        for jj in range(2):
                            nc.tensor.matmul(
                                qk[:, ts(jj, 512)],
                                lhsT=xTr[:, ts(k, 128)],
                                rhs=xTr[:, ds(c * 1024 + jj * 512, 512)],
                                start=True,
                                stop=True,
                            )
                        if k // 8 == c:
                            # kill the score diagonal: S[t,t] += -4e4
                            cw = k * 128 - c * 1024
                            nc.vector.scalar_tensor_tensor(
                                out=qk[:, ds(cw, 128)],
                                in0=ident[:],
                                scalar=DIAG_KILL,
                                in1=qk[:, ds(cw, 128)],
                                op0=MULT,
                                op1=ADD,
                            )
                        nc.scalar.activation(
                            out=Pk[:, ts(c, 1024)],
                            in_=qk[:],
                            func=EXP,
                            bias=negsq[:, k : k + 1],
                            scale=1.0,
                            accum_out=s1[:, 2 * k + c : 2 * k + c + 1],
                        )
                    for j in range(NJ):
                        nc.tensor.matmul(
                            av[j][:],
                            lhsT=vt[:, ts(k, 128)],
                            rhs=Pk[:, ts(j, 512)],
                            start=(k == 0),
                            stop=(k == KB - 1),
                        )

                # ---- denominator: den = 1 + sum of the 32 partial sums
                s12 = smallp.tile([128, KB], F32, tag="s12")
                nc.vector.tensor_reduce(
                    out=s12[:],
                    in_=s1[:].rearrange("p (k c) -> p k c", c=2),
                    axis=AX_X,
                    op=ADD,
                )
                den = smallp.tile([128, KB], F32, tag="den")
                nc.vector.tensor_scalar_add(den[:], s12[:], 1.0)
                r = smallp.tile([128, KB], F32, tag="r")
                nc.vector.reciprocal(r[:], den[:])

                # ---- drain AV groups: transpose corr^T back and combine
                out_nat = iop.tile([128, S], F32, tag="out_nat")
                for j in range(NJ):
                    corrT = iop.tile([128, 512], F32, tag="corrT")
                    nc.vector.tensor_copy(out=corrT[:], in_=av[j][:])
                    for i2 in range(4):
                        i = j * 4 + i2
                        tp2 = qkps.tile([128, 128], F32, tag="qk")
                        nc.tensor.transpose(tp2[:], corrT[:, ts(i2, 128)], ident[:])
                        t1 = t1p.tile([128, 128], F32, tag="t1")
                        nc.vector.scalar_tensor_tensor(
                            out=t1[:],
                            in0=tp2[:],
                            scalar=ru[:, i : i + 1],
                            in1=x_nat[:, ts(i, 128)],
                            op0=MULT,
                            op1=ADD,
                        )
                        nc.vector.tensor_scalar_mul(
                            out_nat[:, ts(i, 128)], t1[:], r[:, i : i + 1]
                        )

                nc.sync.dma_start(
                    out=o_dram.ap()[b].rearrange("(k p) d -> p k d", p=128),
                    in_=out_nat[:].rearrange("p (k d) -> p k d", d=128),
                )

            if loop_n:
                with tc.For_i(
                    0,
                    loop_n,
                    1,
                    hint_engines=(mybir.EngineType.PE, mybir.EngineType.DVE),
                ):
                    for b in range(BPC):
                        batch_body(b)
            else:
                for b in range(BPC):
                    batch_body(b)
    nc.compile()
    return nc


_PROGRAM = None


def _get_program():
    global _PROGRAM
    if _PROGRAM is None:
        _PROGRAM = build_program()
    return _PROGRAM


def run(hidden_states, trace=False, trace_kwargs=None):
    hs = np.ascontiguousarray(np.asarray(hidden_states, dtype=np.float32))
    assert hs.shape == (B, S, D), hs.shape
    nc = _get_program()
    in_maps = [
        {"x": np.ascontiguousarray(hs[c * BPC : (c + 1) * BPC])}
        for c in range(NCORES)
    ]
    res = run_bass_kernel_spmd(
        nc,
        in_maps,
        core_ids=list(range(NCORES)),
        trace=trace,
        **(trace_kwargs or {}),
    )
    out = np.concatenate([r["out"] for r in res.results], axis=0)
    return out, res


def kernel(hidden_states):
    out, _ = run(hidden_states, trace=False)
    return (out, None)
